# revision 2
# baseline (speedup 1.0000x reference)
"""Box-attention kernel v6d for Trainium2 (Bass/Tile), SPMD over 8 NeuronCores.

Problem: per-(batch, h, w) pixel attention over 32 boxes:
  S[i,j] = <q[i,:,p], k[j,:,p]>/8 ; W = softmax_j S ; delta[i,c,p] = sum_j W[i,j] v[j,c,p]

Sharding: core = 2*b + h_half; each core owns (b = core//2, h in [40*(core%2), +40)).

History: v5 (258us) was PE weight-load-ISSUE bound: 6400 (LDWEIGHTS+MATMUL)
pairs at ~40ns. HW microbenchmarks showed the pair floor is ~33-36ns nearly
independent of weight width and N (<=65), so the win is PAIR COUNT. A first
attempt (block-diagonal k materialized via scattered 64B DMA writes) drowned
the Sync engine in descriptor generation (204k x 64B transfers at 1/4 line
rate). v6c gets 2 pixels per matmul with NO zero-padding and fully
contiguous DMAs:

 - scores: pixels p = 2t+a; pair half a hosts pixels (a, 2+a) side by side:
   lhsT = [k_pA | k_pB] [64, 64] (compact), rhs = [q_pA | q_pB], out [64,64]
   diagonal 32-quadrants valid, off-diag = cross-pixel garbage.
   16 MMs per 32-px supergroup.
 - softmax: TWO strided full-lane ACT exp calls per supergroup (one per
   column-quadrant parity), each with a per-partition bias vector adding
   -160 (pre-scaled: -20) to the off-diagonal quadrants, so exp() turns the
   garbage into ~3e-7 -- effectively zero.
 - delta: E [64,64] blocks are then block-diagonal-with-eps, so 2 pixels per
   delta MM too: lhsT = E[64a:+64, 64g:+64], rhs = v pair [64, 65] (ones
   channel = softmax denominator). 16 MMs per supergroup.
 - PE pairs per supergroup: 16 + 16 = 32 (vs 64 in v5).
 - supergroup = 32 px = 8 groups; chunk = 128 px = 4 supergroups.
 - PSUM: eb [128,512] x2 + (ob0 [128,390] + ob1 [128,130]) x3 = 8 banks.
 - fp16 I/O (52.4 MB/core ~= 143us DMA floor), 4 contiguous DMAs per chunk.
"""

import sys

import numpy as np

try:
    import concourse.bass as bass
except ImportError:  # fresh grading dir: point at the in-container repo
    for p in ("/opt/trn_rl_repo", "/root/.axon_site/_ro/trn_rl_repo"):
        if p not in sys.path:
            sys.path.insert(0, p)
    import concourse.bass as bass

from contextlib import ExitStack

import concourse.bacc as bacc
import concourse.tile as tile
from concourse import mybir
from concourse.bass_utils import run_bass_kernel_spmd

NB, B, C, H, W = 32, 4, 64, 80, 80
HPC = H // 2  # 40 h rows per core
NHB, NWB = 5, 5
NCHUNK = 25
IOBUFS = 6
BIAS = 160.0
F16 = mybir.dt.float16
F32 = mybir.dt.float32
EXP = mybir.ActivationFunctionType.Exp

_CACHE = {}


def build_nc():
    nc = bacc.Bacc()
    q = nc.declare_dram_parameter("q", [NCHUNK, 128, 2048], F16, isOutput=False)
    k = nc.declare_dram_parameter("k", [NCHUNK, 128, 2048], F16, isOutput=False)
    v = nc.declare_dram_parameter("v", [NCHUNK, 128, 2080], F16, isOutput=False)
    bz = nc.declare_dram_parameter("bz", [128, 2], F32, isOutput=False)
    o = nc.declare_dram_parameter("o", [NCHUNK, 128, 2048], F16, isOutput=True)
    qv, kv, vv, ov = q[:], k[:], v[:], o[:]

    with tile.TileContext(nc) as tc, ExitStack() as ctx:
        io = ctx.enter_context(tc.tile_pool(name="io", bufs=IOBUFS))
        sm = ctx.enter_context(tc.tile_pool(name="sm", bufs=3))
        ep = ctx.enter_context(tc.tile_pool(name="ep", bufs=2, space="PSUM"))
        op = ctx.enter_context(tc.tile_pool(name="op", bufs=3, space="PSUM"))

        tbz = io.tile([128, 2], F32, tag="tbz", name="tbz", bufs=1)
        nc.sync.dma_start(out=tbz, in_=bz[:])

        chunk_tiles = {}

        def emit_chunk_dma(cb):
            tq = io.tile([128, 4, 8, 64], F16, tag="tq", name="tq")
            tk = io.tile([128, 4, 8, 64], F16, tag="tk", name="tk")
            tv = io.tile([128, 4, 8, 65], F16, tag="tv", name="tv")
            to = io.tile([128, 4, 8, 64], F16, tag="to", name="to")
            nc.sync.dma_start(out=tq.rearrange("p s g x -> p (s g x)"),
                              in_=qv[cb])
            nc.sync.dma_start(out=tk.rearrange("p s g x -> p (s g x)"),
                              in_=kv[cb])
            nc.sync.dma_start(out=tv.rearrange("p s g c -> p (s g c)"),
                              in_=vv[cb])
            chunk_tiles[cb] = (tq, tk, tv, to)

        def emit_chunk_out(cb):
            (_, _, _, to) = chunk_tiles.pop(cb)
            nc.sync.dma_start(out=ov[cb],
                              in_=to.rearrange("p s g c -> p (s g c)"))

        def emit_scores(sg):
            cb, s = sg["cb"], sg["s"]
            tq, tk, _, _ = chunk_tiles[cb]
            eb = ep.tile([128, 512], F32, tag="eb", name="eb")
            for g in range(8):
                for a in range(2):
                    nc.tensor.matmul(
                        out=eb[64 * a:64 * a + 64, 64 * g:64 * g + 64],
                        lhsT=tk[64 * a:64 * a + 64, s, g, :],
                        rhs=tq[64 * a:64 * a + 64, s, g, :],
                        start=True, stop=True,
                        tile_position=(64 * a, 64 * a))
            sg["eb"] = eb

        def emit_softmax_delta(sg):
            cb, s = sg["cb"], sg["s"]
            _, _, tv, to = chunk_tiles[cb]
            eb = sg.pop("eb")
            E = sm.tile([128, 512], F16, tag="E", name="E")
            ebv = eb.rearrange("p (g d x) -> p g d x", g=8, d=2)
            Ev = E.rearrange("p (g d x) -> p g d x", g=8, d=2)
            for d in range(2):
                nc.scalar.activation(Ev[:, :, d, :], ebv[:, :, d, :], EXP,
                                     scale=0.125, bias=tbz[:, d:d + 1])
            ob0 = op.tile([128, 390], F32, tag="ob0", name="ob0")
            ob1 = op.tile([128, 130], F32, tag="ob1", name="ob1")
            obs = ((ob0, 0, 6), (ob1, 6, 2))
            for g in range(8):
                ob, u = (ob0, g) if g < 6 else (ob1, g - 6)
                for a in range(2):
                    nc.tensor.matmul(
                        out=ob[64 * a:64 * a + 64, 65 * u:65 * u + 65],
                        lhsT=E[64 * a:64 * a + 64, 64 * g:64 * g + 64],
                        rhs=tv[64 * a:64 * a + 64, s, g, :],
                        start=True, stop=True,
                        tile_position=(64 * a, 64 * a))
            rden = sm.tile([128, 8], F32, tag="rden", name="rden")
            for ob, u0, ng in obs:
                obv = ob[:, 0:65 * ng].rearrange("p (u c) -> p u c", u=ng)
                nc.vector.reciprocal(rden[:, u0:u0 + ng], obv[:, :, 64])
                nc.vector.tensor_mul(
                    to[:, s, u0:u0 + ng, :],
                    obv[:, :, 0:64],
                    rden[:, u0:u0 + ng].unsqueeze(2).broadcast_to(
                        (128, ng, 64)))

        sgs = [{"cb": cb, "s": s} for cb in range(NCHUNK) for s in range(4)]
        emit_chunk_dma(0)
        emit_chunk_dma(1)
        emit_chunk_dma(2)
        pending = None
        for sg in sgs:
            if sg["s"] == 0 and sg["cb"] + 3 < NCHUNK:
                emit_chunk_dma(sg["cb"] + 3)
            emit_scores(sg)
            if pending is not None:
                emit_softmax_delta(pending)
                if pending["s"] == 3:
                    emit_chunk_out(pending["cb"])
            pending = sg
        emit_softmax_delta(pending)
        emit_chunk_out(pending["cb"])
    nc.compile()
    return nc


def _get_nc():
    if "nc" not in _CACHE:
        _CACHE["nc"] = build_nc()
    return _CACHE["nc"]


# pixel mapping within a chunk (8 hc x 16 u):
#   r = hc % 2, eh = hc // 2, uh2 = u // 8, gw = (u % 8) // 4, p = u % 4
#   sg s = 2*r + uh2; group g = 2*eh + gw; p = 2*t + a

def _pack_q(a):
    # [32 i, 64 c, 40, 80] -> [25, 128(a c), (s g t i)] f16
    t = np.asarray(a).reshape(NB, C, NHB, 4, 2, NWB, 2, 2, 2, 2)
    t = t.transpose(2, 5, 9, 1, 4, 6, 3, 7, 8, 0).astype(np.float16)
    return t.reshape(NCHUNK, 128, 2048)


_pack_k = _pack_q


def _pack_v(a):
    # -> [25, 128(a t j), (s g c65)] f16 with ones channel
    t = np.asarray(a).reshape(NB, C, NHB, 4, 2, NWB, 2, 2, 2, 2)
    t = t.transpose(2, 5, 9, 8, 0, 4, 6, 3, 7, 1).astype(np.float16)
    t = t.reshape(NCHUNK, 128, 4, 8, C)
    ones = np.ones(t.shape[:-1] + (1,), np.float16)
    return np.concatenate([t, ones], axis=-1).reshape(NCHUNK, 128, 2080)


def _unpack_o(oh):
    # [25, 128(a t i), (s g c)] f16 -> [32 i, 64 c, 40, 80] f32
    t = oh.reshape(NHB, NWB, 2, 2, NB, 2, 2, 4, 2, C).astype(np.float32)
    t = t.transpose(4, 9, 0, 7, 5, 1, 6, 8, 3, 2)  # i c hb eh r wb uh2 gw t a
    return t.reshape(NB, C, HPC, W)


def _bias_const():
    # bias[row, d] applies to column-quadrants with t' == d:
    # -BIAS (pre-scaled by 0.125 -> -20) where row-parity t != d
    bz = np.zeros((128, 2), np.float32)
    for row in range(128):
        t = (row // 32) % 2
        bz[row, 1 - t] = -BIAS * 0.125
    return bz


def kernel(q_big, k_big, v_big, **run_kwargs):
    nc = _get_nc()
    bz = _bias_const()
    in_maps = []
    for core in range(8):
        b, h0 = core // 2, HPC * (core % 2)
        sl = np.s_[:, b, :, h0:h0 + HPC, :]
        in_maps.append(
            {
                "q": _pack_q(q_big[sl]),
                "k": _pack_k(k_big[sl]),
                "v": _pack_v(v_big[sl]),
                "bz": bz,
            }
        )
    res = run_bass_kernel_spmd(nc, in_maps, list(range(8)), **run_kwargs)
    out = np.empty((NB, B, C, H, W), np.float32)
    for core in range(8):
        b, h0 = core // 2, HPC * (core % 2)
        out[:, b, :, h0:h0 + HPC, :] = _unpack_o(res.results[core]["o"])
    if run_kwargs:
        kernel.last_results = res
    return out


# revision 3
# speedup vs baseline: 1.0415x; 1.0415x over previous
"""Box-attention kernel v6 for Trainium2 (Bass/Tile), SPMD over 8 NeuronCores.

Problem: per-(batch, h, w) pixel attention over 32 boxes:
  S[i,j] = <q[i,:,p], k[j,:,p]>/8 ; W = softmax_j S ; delta[i,c,p] = sum_j W[i,j] v[j,c,p]

Sharding: core = 2*b + h_half; each core owns (b = core//2, h in [40*(core%2), +40)).

Measured: ~182-188us HW exec (rel err 5.9e-4), vs 258us for v5. This is the
DMA wall: a DMA-only microbenchmark of the identical 52.6 MB/core fp16 byte
pattern on 8 concurrent cores runs in 188us (~280 GB/s effective per core,
8-core HBM contention), so compute is fully hidden.

History: v5 (258us) was PE weight-load-ISSUE bound: 6400 (LDWEIGHTS+MATMUL)
pairs at ~40ns. HW microbenchmarks showed the pair floor is ~33-36ns nearly
independent of weight width and N (<=65), so the win is PAIR COUNT. A first
attempt (block-diagonal k materialized via scattered 64B DMA writes) drowned
the Sync engine in descriptor generation (204k x 64B transfers at 1/4 line
rate). v6 gets 2 pixels per matmul with NO zero-padding and fully
contiguous DMAs:

 - scores: pixels p = 2t+a; pair half a hosts pixels (a, 2+a) side by side:
   lhsT = [k_pA | k_pB] [64, 64] (compact), rhs = [q_pA | q_pB], out [64,64]
   diagonal 32-quadrants valid, off-diag = cross-pixel garbage.
   16 MMs per 32-px supergroup.
 - softmax: TWO strided full-lane ACT exp calls per supergroup (one per
   column-quadrant parity), each with a per-partition bias vector adding
   -160 (pre-scaled: -20) to the off-diagonal quadrants, so exp() turns the
   garbage into ~3e-7 -- effectively zero.
 - delta: E [64,64] blocks are then block-diagonal-with-eps, so 2 pixels per
   delta MM too: lhsT = E[64a:+64, 64g:+64], rhs = v pair [64, 65] (ones
   channel = softmax denominator). 16 MMs per supergroup.
 - PE pairs per supergroup: 16 + 16 = 32 (vs 64 in v5).
 - supergroup = 32 px = 8 groups; chunk = 128 px = 4 supergroups.
 - PSUM: eb [128,512] x2 + (ob0 [128,390] + ob1 [128,130]) x3 = 8 banks.
 - fp16 I/O (52.4 MB/core ~= 143us DMA floor), 4 contiguous DMAs per chunk.
"""

import sys

import numpy as np

try:
    import concourse.bass as bass
except ImportError:  # fresh grading dir: point at the in-container repo
    for p in ("/opt/trn_rl_repo", "/root/.axon_site/_ro/trn_rl_repo"):
        if p not in sys.path:
            sys.path.insert(0, p)
    import concourse.bass as bass

from contextlib import ExitStack

import concourse.bacc as bacc
import concourse.tile as tile
from concourse import mybir
from concourse.bass_utils import run_bass_kernel_spmd

NB, B, C, H, W = 32, 4, 64, 80, 80
HPC = H // 2  # 40 h rows per core
NHB, NWB = 5, 5
NCHUNK = 25
IOBUFS = 6
BIAS = 160.0
F16 = mybir.dt.float16
F32 = mybir.dt.float32
EXP = mybir.ActivationFunctionType.Exp

_CACHE = {}


def build_nc():
    nc = bacc.Bacc()
    q = nc.declare_dram_parameter("q", [NCHUNK, 128, 2048], F16, isOutput=False)
    k = nc.declare_dram_parameter("k", [NCHUNK, 128, 2048], F16, isOutput=False)
    v = nc.declare_dram_parameter("v", [NCHUNK, 128, 2080], F16, isOutput=False)
    bz = nc.declare_dram_parameter("bz", [128, 2], F32, isOutput=False)
    o = nc.declare_dram_parameter("o", [NCHUNK, 128, 2048], F16, isOutput=True)
    qv, kv, vv, ov = q[:], k[:], v[:], o[:]

    with tile.TileContext(nc) as tc, ExitStack() as ctx:
        io = ctx.enter_context(tc.tile_pool(name="io", bufs=IOBUFS))
        sm = ctx.enter_context(tc.tile_pool(name="sm", bufs=3))
        ep = ctx.enter_context(tc.tile_pool(name="ep", bufs=2, space="PSUM"))
        op = ctx.enter_context(tc.tile_pool(name="op", bufs=3, space="PSUM"))

        tbz = io.tile([128, 2], F32, tag="tbz", name="tbz", bufs=1)
        nc.sync.dma_start(out=tbz, in_=bz[:])

        chunk_tiles = {}

        def emit_chunk_dma(cb):
            tq = io.tile([128, 4, 8, 64], F16, tag="tq", name="tq")
            tk = io.tile([128, 4, 8, 64], F16, tag="tk", name="tk")
            tv = io.tile([128, 4, 8, 65], F16, tag="tv", name="tv")
            to = io.tile([128, 4, 8, 64], F16, tag="to", name="to")
            nc.sync.dma_start(out=tq.rearrange("p s g x -> p (s g x)"),
                              in_=qv[cb])
            nc.sync.dma_start(out=tk.rearrange("p s g x -> p (s g x)"),
                              in_=kv[cb])
            nc.sync.dma_start(out=tv.rearrange("p s g c -> p (s g c)"),
                              in_=vv[cb])
            chunk_tiles[cb] = (tq, tk, tv, to)

        def emit_chunk_out(cb):
            (_, _, _, to) = chunk_tiles.pop(cb)
            nc.sync.dma_start(out=ov[cb],
                              in_=to.rearrange("p s g c -> p (s g c)"))

        def emit_scores(sg):
            cb, s = sg["cb"], sg["s"]
            tq, tk, _, _ = chunk_tiles[cb]
            eb = ep.tile([128, 512], F32, tag="eb", name="eb")
            for g in range(8):
                for a in range(2):
                    nc.tensor.matmul(
                        out=eb[64 * a:64 * a + 64, 64 * g:64 * g + 64],
                        lhsT=tk[64 * a:64 * a + 64, s, g, :],
                        rhs=tq[64 * a:64 * a + 64, s, g, :],
                        start=True, stop=True,
                        tile_position=(64 * a, 64 * a))
            sg["eb"] = eb

        def emit_softmax_delta(sg):
            cb, s = sg["cb"], sg["s"]
            _, _, tv, to = chunk_tiles[cb]
            eb = sg.pop("eb")
            E = sm.tile([128, 512], F16, tag="E", name="E")
            ebv = eb.rearrange("p (g d x) -> p g d x", g=8, d=2)
            Ev = E.rearrange("p (g d x) -> p g d x", g=8, d=2)
            for d in range(2):
                nc.scalar.activation(Ev[:, :, d, :], ebv[:, :, d, :], EXP,
                                     scale=0.125, bias=tbz[:, d:d + 1])
            ob0 = op.tile([128, 390], F32, tag="ob0", name="ob0")
            ob1 = op.tile([128, 130], F32, tag="ob1", name="ob1")
            obs = ((ob0, 0, 6), (ob1, 6, 2))
            for g in range(8):
                ob, u = (ob0, g) if g < 6 else (ob1, g - 6)
                for a in range(2):
                    nc.tensor.matmul(
                        out=ob[64 * a:64 * a + 64, 65 * u:65 * u + 65],
                        lhsT=E[64 * a:64 * a + 64, 64 * g:64 * g + 64],
                        rhs=tv[64 * a:64 * a + 64, s, g, :],
                        start=True, stop=True,
                        tile_position=(64 * a, 64 * a))
            rden = sm.tile([128, 8], F32, tag="rden", name="rden")
            for ob, u0, ng in obs:
                obv = ob[:, 0:65 * ng].rearrange("p (u c) -> p u c", u=ng)
                nc.vector.reciprocal(rden[:, u0:u0 + ng], obv[:, :, 64])
                nc.vector.tensor_mul(
                    to[:, s, u0:u0 + ng, :],
                    obv[:, :, 0:64],
                    rden[:, u0:u0 + ng].unsqueeze(2).broadcast_to(
                        (128, ng, 64)))

        sgs = [{"cb": cb, "s": s} for cb in range(NCHUNK) for s in range(4)]
        emit_chunk_dma(0)
        emit_chunk_dma(1)
        emit_chunk_dma(2)
        pending = None
        for sg in sgs:
            if sg["s"] == 0 and sg["cb"] + 3 < NCHUNK:
                emit_chunk_dma(sg["cb"] + 3)
            emit_scores(sg)
            if pending is not None:
                emit_softmax_delta(pending)
                if pending["s"] == 3:
                    emit_chunk_out(pending["cb"])
            pending = sg
        emit_softmax_delta(pending)
        emit_chunk_out(pending["cb"])
    nc.compile()
    return nc


def _get_nc():
    if "nc" not in _CACHE:
        _CACHE["nc"] = build_nc()
    return _CACHE["nc"]


# pixel mapping within a chunk (8 hc x 16 u):
#   r = hc % 2, eh = hc // 2, uh2 = u // 8, gw = (u % 8) // 4, p = u % 4
#   sg s = 2*r + uh2; group g = 2*eh + gw; p = 2*t + a

def _pack_q(a):
    # [32 i, 64 c, 40, 80] -> [25, 128(a c), (s g t i)] f16
    t = np.asarray(a).reshape(NB, C, NHB, 4, 2, NWB, 2, 2, 2, 2)
    t = t.transpose(2, 5, 9, 1, 4, 6, 3, 7, 8, 0).astype(np.float16)
    return t.reshape(NCHUNK, 128, 2048)


_pack_k = _pack_q


def _pack_v(a):
    # -> [25, 128(a t j), (s g c65)] f16 with ones channel
    t = np.asarray(a).reshape(NB, C, NHB, 4, 2, NWB, 2, 2, 2, 2)
    t = t.transpose(2, 5, 9, 8, 0, 4, 6, 3, 7, 1).astype(np.float16)
    t = t.reshape(NCHUNK, 128, 4, 8, C)
    ones = np.ones(t.shape[:-1] + (1,), np.float16)
    return np.concatenate([t, ones], axis=-1).reshape(NCHUNK, 128, 2080)


def _unpack_o(oh):
    # [25, 128(a t i), (s g c)] f16 -> [32 i, 64 c, 40, 80] f32
    t = oh.reshape(NHB, NWB, 2, 2, NB, 2, 2, 4, 2, C).astype(np.float32)
    t = t.transpose(4, 9, 0, 7, 5, 1, 6, 8, 3, 2)  # i c hb eh r wb uh2 gw t a
    return t.reshape(NB, C, HPC, W)


def _bias_const():
    # bias[row, d] applies to column-quadrants with t' == d:
    # -BIAS (pre-scaled by 0.125 -> -20) where row-parity t != d
    bz = np.zeros((128, 2), np.float32)
    for row in range(128):
        t = (row // 32) % 2
        bz[row, 1 - t] = -BIAS * 0.125
    return bz


def kernel(q_big, k_big, v_big, **run_kwargs):
    nc = _get_nc()
    bz = _bias_const()
    in_maps = []
    for core in range(8):
        b, h0 = core // 2, HPC * (core % 2)
        sl = np.s_[:, b, :, h0:h0 + HPC, :]
        in_maps.append(
            {
                "q": _pack_q(q_big[sl]),
                "k": _pack_k(k_big[sl]),
                "v": _pack_v(v_big[sl]),
                "bz": bz,
            }
        )
    res = run_bass_kernel_spmd(nc, in_maps, list(range(8)), **run_kwargs)
    out = np.empty((NB, B, C, H, W), np.float32)
    for core in range(8):
        b, h0 = core // 2, HPC * (core % 2)
        out[:, b, :, h0:h0 + HPC, :] = _unpack_o(res.results[core]["o"])
    if run_kwargs:
        kernel.last_results = res
    return out
